# revision 17
# baseline (speedup 1.0000x reference)
"""Trainium2 Bass kernel for nn_DGG_LearnableK_Small.

The reference collapses analytically:
  - softmax over a size-1 axis == 1, so log_p == 0 and edge_prob == 1/N exactly
    (for any temp); stable argsort of a constant row is the identity
    permutation, so idxs[b,i,j] = j and the scatter/gather permutations are
    identity.  idx is therefore an input-independent constant: the device
    emits one replicated iota tile and the host broadcast is the gather.
  - adj_hard[b,i,j] = sigmoid(x_support[j] + 7*k[b,i]) where
    k = (relu(x @ W_mu1 + b_mu1) @ W_mu2 + b_mu2) @ W_kp + b_kp,
    x_support[j] = 2 - 7j.  sigmoid underflows to exactly 0.0f for j >= 16
    at any plausible shift; CUT=32 columns are computed (2x margin), the
    rest of adj is zeros assembled on the host.

Host folding: wv7 = W_mu2 @ (7*W_kp) collapses the linear tail.  The mixed
signs of wv7 fold into the first layer:  with W1f = W_mu1 * wv7 (natural,
signed, per-column scale) and b1f = b_mu1 * wv7, columns permuted
positive-wv7-first,

  7*k + const = cke' + sum_pos max(z_l, -b_l) + sum_neg min(z_l, -b_l),
  z = x @ W1f,   cke' = cke + sum(b1f)

because for w < 0, w*relu(u+b) = min((u+b)*w, 0) = min(uw, -bw) + bw.  The
bias therefore never has to be added on-device: each block is one fused
scalar_tensor_tensor ((z mult 1) max/min -b) whose accum_out row-reduces
in the same pass.

Per core (1024 rows, 8 row-chunks of 128), instruction-count-minimized
(a ~10us Bacc/NEFF envelope, ~600ns per DMA instruction, and 0.1-0.3us
per-compute-instruction overheads dominate at this scale):
  PE:   per chunk one bf16 matmul (lhsT = xT chunk, rhs = W1f).
  DVE:  per chunk two fused max/min+accum passes over the PSUM tile.
  GpSimd: the [128,1]+[128,1] shift combines (SBUF-only engine), plus
        idx = int32 iota [128,16] (channel_multiplier=16); host reshapes
        to the identity row and broadcasts as the gather step.
  ACT:  per chunk one Sigmoid over iof2[p,j] = -7j + cke' with bias = the
        combined shift; adj rides the ACT-sequencer DMA ring in-order.
  DMA:  inputs split across the SP ring (xT) and GpSimd ring (W/-b) in
        parallel; sigmoid input iota on the ACT ring.
"""

import os

import numpy as np

B, N, D, L = 4, 2048, 128, 256
NCORES = 8
ROWS = B * N          # 8192
RPC = ROWS // NCORES  # 1024 rows per core
P = 128
RCHUNKS = RPC // P    # 8
INTERVAL = 7.0
HS_START = 2.0
CUT = 32              # adj columns actually computed (rest stay 0)
XCOLS = RPC           # xT tensor [128, 1024]
PWC = 2 * L           # [W1f | -b1f] tensor [128, 512]

_CACHE = {}

# Results of the last device run (exec time etc.) for the local test harness.
LAST_RESULTS = None


def _build_nc(lp):
    import concourse.bacc as bacc
    import concourse.mybir as mybir
    from concourse.tile import TileContext

    f32 = mybir.dt.float32
    bf16 = mybir.dt.bfloat16
    i32 = mybir.dt.int32
    AF = mybir.ActivationFunctionType
    OP = mybir.AluOpType

    # Bacc (not plain Bass): its compile() legalizes semaphore waits for the
    # TRN2 one-wait-per-instruction constraint via event semaphores.
    nc = bacc.Bacc(None, target_bir_lowering=False, debug=False)
    px = nc.declare_dram_parameter("px", [P, XCOLS], bf16, isOutput=False)
    pw = nc.declare_dram_parameter("pw", [P, PWC], bf16, isOutput=False)
    pkf = nc.declare_dram_parameter("pkf", [P, CUT], f32, isOutput=False)
    adj = nc.declare_dram_parameter("adj", [RPC, CUT], f32, isOutput=True)
    idx = nc.declare_dram_parameter("idx", [P, N // P], i32, isOutput=True)

    with TileContext(nc) as tc:
        with (
            tc.tile_pool(name="const", bufs=1) as cpool,
            tc.tile_pool(name="ps", bufs=1, space="PSUM") as ppool,
            tc.tile_pool(name="wk", bufs=3) as wpool,
        ):
            pkf_sb = cpool.tile([P, CUT], f32, tag="pkf")
            px_sb = cpool.tile([P, XCOLS], bf16, tag="px")
            pw_sb = cpool.tile([P, PWC], bf16, tag="pw")
            # Parallel input rings (only SP/ACT/GpSimd have HWDGE rings):
            # SP carries the contiguous xT tensor in one full-rate
            # descriptor, GpSimd ring carries W/-b, ACT ring the iota.
            nc.sync.dma_start(out=px_sb, in_=px[:])
            nc.gpsimd.dma_start(out=pw_sb, in_=pw[:])
            nc.scalar.dma_start(out=pkf_sb, in_=pkf[:])

            # idx afterwards on the then-idle GpSimd queue; value at [p, j]
            # is 16p + j, so the row-major flatten is the identity row.
            idx_sb = cpool.tile([P, N // P], i32, tag="idx")
            nc.gpsimd.iota(idx_sb, pattern=[[1, N // P]], base=0,
                           channel_multiplier=N // P)
            nc.gpsimd.dma_start(out=idx[:], in_=idx_sb)

            w1_ap = pw_sb[:, 0:L]
            nbp_ap = pw_sb[:, L:L + lp]
            nbn_ap = pw_sb[:, L + lp:2 * L]

            fk = cpool.tile([P, RCHUNKS * CUT], f32, tag="fk")
            for c in range(RCHUNKS):
                z = ppool.tile([P, L], f32, tag=f"z{c}")
                nc.tensor.matmul(
                    z,
                    lhsT=px_sb[:, c * P:(c + 1) * P],
                    rhs=w1_ap,
                    start=True,
                    stop=True,
                )
                junk = wpool.tile([P, L], f32, tag="junk")
                ab = wpool.tile([P, 2], f32, tag="ab")
                nc.vector.scalar_tensor_tensor(
                    junk[:, 0:lp], z[:, 0:lp], 1.0, nbp_ap,
                    OP.mult, OP.max, accum_out=ab[:, 0:1],
                )
                nc.vector.scalar_tensor_tensor(
                    junk[:, lp:L], z[:, lp:L], 1.0, nbn_ap,
                    OP.mult, OP.min, accum_out=ab[:, 1:2],
                )
                sc = wpool.tile([P, 1], f32, tag="sc")
                nc.gpsimd.tensor_tensor(sc, ab[:, 0:1], ab[:, 1:2], OP.add)
                nc.scalar.activation(
                    fk[:, c * CUT:(c + 1) * CUT],
                    pkf_sb,
                    AF.Sigmoid,
                    bias=sc,
                    scale=1.0,
                )
            # adj goes out on the ACT-sequencer HWDGE ring, in-order after
            # the last sigmoid (no cross-engine semaphore on the tail).
            nc.scalar.dma_start(
                out=adj.rearrange("(rc p) c -> p rc c", p=P),
                in_=fk.rearrange("p (rc c) -> p rc c", c=CUT),
            )

    nc.compile()
    return nc


def kernel(**inputs):
    global LAST_RESULTS
    import ml_dtypes
    from concourse.bass_utils import run_bass_kernel_spmd

    bf16 = ml_dtypes.bfloat16

    x = np.ascontiguousarray(np.asarray(inputs["x"], dtype=np.float32))
    W1 = np.asarray(inputs["W_mu1"], dtype=np.float32)
    b1v = np.asarray(inputs["b_mu1"], dtype=np.float32)
    W2 = np.asarray(inputs["W_mu2"], dtype=np.float32)
    b2v = np.asarray(inputs["b_mu2"], dtype=np.float32)
    Wkp = np.asarray(inputs["W_kp"], dtype=np.float32)
    bkp = np.asarray(inputs["b_kp"], dtype=np.float32)

    # Host-side folding of the linear tail (replicated across cores).
    wv7 = (W2.astype(np.float64) @ (INTERVAL * Wkp[:, 0].astype(np.float64)))
    cke = HS_START + INTERVAL * float(
        b2v.astype(np.float64) @ Wkp[:, 0].astype(np.float64)
        + np.float64(bkp[0]))
    W1f = W1.astype(np.float64) * wv7[None, :]
    b1f = b1v.astype(np.float64) * wv7
    pos = wv7 > 0
    perm = np.concatenate([np.where(pos)[0], np.where(~pos)[0]])
    lp = int(pos.sum())
    W1p = np.ascontiguousarray(W1f[:, perm]).astype(np.float32)
    b1p = np.ascontiguousarray(b1f[perm]).astype(np.float32)
    # max(z+b,0) = max(z,-b) + b on-device; sum(b) rides in the constant.
    negb = (-b1p).astype(bf16)
    ckeb = cke + float(np.sum(-negb.astype(np.float64)))

    key = ("nc", lp)
    if key not in _CACHE:
        _CACHE[key] = _build_nc(lp)
    nc = _CACHE[key]

    pkf = np.ascontiguousarray(
        np.broadcast_to(
            (ckeb - INTERVAL * np.arange(CUT, dtype=np.float64)).astype(
                np.float32), (P, CUT)))

    x_flat = x.reshape(ROWS, D)
    pw = np.empty((P, PWC), dtype=bf16)
    pw[:, 0:L] = W1p.astype(bf16)
    pw[:, L:2 * L] = negb[None, :]

    in_maps = []
    for c in range(NCORES):
        px = np.ascontiguousarray(
            x_flat[c * RPC:(c + 1) * RPC].T).astype(bf16)
        in_maps.append({"px": px, "pw": pw, "pkf": pkf})

    try:
        res = run_bass_kernel_spmd(nc, in_maps, list(range(NCORES)))
    except ModuleNotFoundError:
        # BASS_TRACE was set in an environment without the axon NTFF hook
        # module; retry with tracing forced off.
        os.environ["BASS_NEVER_TRACE"] = "1"
        res = run_bass_kernel_spmd(nc, in_maps, list(range(NCORES)))
    LAST_RESULTS = res

    adj_full = np.zeros((ROWS, N), dtype=np.float32)
    for c in range(NCORES):
        adj_full[c * RPC:(c + 1) * RPC, 0:CUT] = res.results[c]["adj"]
    idx_row = res.results[0]["idx"].reshape(N)
    idx_full = np.broadcast_to(idx_row, (B, N, N)).copy()

    return adj_full.reshape(B, N, N), idx_full
